# revision 16
# baseline (speedup 1.0000x reference)
"""Trainium2 Bass kernel for nn_CrossAttention_82429012345074.

8-head self-attention, B=2, N=4096, d_model=512, 8 heads x 64 dim.

Sharding: one head per NeuronCore (8 heads / 8 cores) — tensor parallel:
to_q/k/v column-parallel (each core gets its head's 64 rows of Wq/Wk/Wv),
to_out row-parallel (each core gets its head's 64 columns of Wo and emits a
partial [tok, 512] output). The unshard step sums the 8 partials + bias on
host.

Per-core device kernel (all matmuls in bf16, fp32 accumulation):
  xT = dma(x pre-transposed on host)           # [512f, 8192t] in 4 chunks
  q/k proj: COL-TILED pair — wq into PE cols 0-63, wk into cols 64-127,
     both streaming the same xT chunk concurrently.  qk_ps[0:64]=q,
     [64:128]=k.  q is copied to BOTH partition halves of qT2; k is
     split even/odd key-block into kT2's partition halves.
  v     = xT.T @ Wv.T (natural layout)         # [8192, 64] + ones column
  per (batch, 512-query group), chunks of 3 key blocks:
     QK is ROW-TILED: the even/odd key blocks of a pair use PE rows
     0-63 / 64-127 (K=64 each) and run concurrently; the leftover
     single block uses whichever half its parity selects.
     pT = exp(sT * scale)                      # ScalarE, PSUM->SBUF bf16
     o[65, q] += [v|1].T @ pT                  # accumulate over j; row 64 = denom
     oN = o[0:64] (unnormalized, bf16); denom transposed via K=1 matmuls,
     reciprocal'd, applied as per-partition scalar in the projection's
     PSUM->SBUF copy.  Epilogue interleaved into the next group's chunks.

Pipelining: only projection groups 0-1 run before attention starts; the
remaining 14 projection groups (incl. all of batch 1) are interleaved into
the attention chunk stream as single-psum-tile "pieces" so their PE work
fills the slack of the ScalarE(exp)-paced steady state.  The kernel opens
with filler LDWEIGHTS so the PE HAM clock-gate is already warm when the
first real matmul issues.
"""

import sys

sys.path.insert(0, "/opt/trn_rl_repo")

import numpy as np
import ml_dtypes

B, N, D, H, DH = 2, 4096, 512, 8, 64
TOK = B * N            # 8192
NQ = 512               # query-group width
NCH = D // 128         # 4 feature chunks of x
NJB = N // 128         # 32 key blocks per batch
NPAIR = NJB // 2       # 16 key-block pairs per batch
NTB = TOK // 128       # 64 token blocks
NG = TOK // NQ         # 16 query groups (both batches)
JGS = [3] * 10 + [2]   # key-blocks per exp() chunk (sum = 32)
SCALE = DH ** -0.5
N_FILL = 0             # PE filler LDWEIGHTS per chunk (off: they stall the queue)
N_WARM = 48            # filler LDWEIGHTS at kernel start (HAM pre-warm)
# Schraudolph fast-exp constants (DVE path): bf16 bits of exp(s*SCALE) are
# approximated by int16(round(s * SCH_A + SCH_B)); the softmax denominator
# self-normalization cancels most of the sawtooth error (measured 2e-3).
SCH_A = SCALE * 128 * 1.4426950408889634
SCH_B = 16256.0 - 8.0


def build_bass():
    from contextlib import ExitStack

    import concourse.bass as bass
    import concourse.mybir as mybir
    import concourse.tile as tile
    from concourse import bacc

    f32 = mybir.dt.float32
    bf16 = mybir.dt.bfloat16
    i16 = mybir.dt.int16
    EXP = mybir.ActivationFunctionType.Exp
    MULT = mybir.AluOpType.mult
    ADD = mybir.AluOpType.add

    nc = bacc.Bacc("TRN2", target_bir_lowering=False, num_devices=8)
    x_d = nc.dram_tensor("x", [NCH, 128, TOK], bf16, kind="ExternalInput")
    wq_d = nc.dram_tensor("wq", [D, DH], bf16, kind="ExternalInput")
    wk_d = nc.dram_tensor("wk", [D, DH], bf16, kind="ExternalInput")
    wv_d = nc.dram_tensor("wv", [D, DH], bf16, kind="ExternalInput")
    wo_d = nc.dram_tensor("wo", [DH, D], bf16, kind="ExternalInput")
    out_d = nc.dram_tensor("out", [TOK, D], f32, kind="ExternalOutput")

    with tile.TileContext(nc) as tc, ExitStack() as ctx:
        const = ctx.enter_context(tc.tile_pool(name="const", bufs=1))
        sb_p = ctx.enter_context(tc.tile_pool(name="sb_p", bufs=3))
        sb_io = ctx.enter_context(tc.tile_pool(name="sb_io", bufs=3))
        ps_s = ctx.enter_context(tc.tile_pool(name="ps_s", bufs=2, space="PSUM"))
        ps_sm = ctx.enter_context(tc.tile_pool(name="ps_sm", bufs=2, space="PSUM"))

        # Long-lived SBUF tensors
        xT = const.tile([128, NCH, TOK], bf16, name="xT")      # x transposed, 4 chunks
        # qT duplicated in both partition halves (rhs for either QK row-tile)
        qT2 = const.tile([128, TOK], bf16, name="qT2")
        # kT split by key-block parity: partitions 0-63 = even blocks,
        # 64-127 = odd blocks; col = (block>>1)*128 + key
        kT2 = const.tile([128, B, NPAIR * 128], bf16, name="kT2")
        vP = const.tile([128, NTB, DH + 1], bf16, name="vP")   # v blocks + ones col
        oN = const.tile([64, TOK], bf16, name="oN")            # unnormalized attn out^T
        wq = const.tile([128, NCH, DH], bf16, name="wq")
        wk = const.tile([128, NCH, DH], bf16, name="wk")
        wv = const.tile([128, NCH, DH], bf16, name="wv")
        wo = const.tile([64, D], bf16, name="wo")
        warm = const.tile([128, 128], bf16, name="warm")

        nc.vector.memset(warm, 0.0)
        nc.vector.memset(vP[:, :, DH : DH + 1], 1.0)
        ones1 = const.tile([1, 1], f32, name="ones1")
        nc.vector.memset(ones1, 1.0)

        # HAM pre-warm: give the PE ~5us of LDWEIGHTS busy-work while the
        # x DMA lands, so the activity monitor un-gates the clock before the
        # first projection matmul.
        for _w in range(N_WARM):
            nc.tensor.ldweights(weights=warm)

        # Phase 0: weights + x DMA ordered so projection group 0 can start
        # as early as possible (DMA queue is serial): wq/wk, then the first
        # 512-col x slab, then wv for group 0's v-proj, widening slabs after.
        nc.sync.dma_start(out=wq, in_=wq_d[:].rearrange("(c p) d -> p c d", p=128))
        nc.sync.dma_start(out=wk, in_=wk_d[:].rearrange("(c p) d -> p c d", p=128))
        for c in range(NCH):
            nc.sync.dma_start(out=xT[:, c, 0:512], in_=x_d[c, :, 0:512])
        nc.sync.dma_start(out=wv, in_=wv_d[:].rearrange("(c p) d -> p c d", p=128))
        nc.sync.dma_start(out=wo, in_=wo_d[:])
        slabs = [(512, 1024), (1024, 2048)] + [
            (t0, t0 + 2048) for t0 in range(2048, TOK, 2048)
        ]
        for t0, t1 in slabs:
            for c in range(NCH):
                nc.sync.dma_start(
                    out=xT[:, c, t0:t1],
                    in_=x_d[c, :, t0:t1],
                )

        # ---- projection emitters ---------------------------------------
        def emit_proj_qk(g):
            """col-tiled q|k projection for query group g + layout copies"""
            b, qg, t0 = g // (N // NQ), g % (N // NQ), g * NQ
            qk = ps_sm.tile([128, NQ], f32, tag="o", name="qk")
            for c in range(NCH):
                nc.tensor.matmul(
                    qk[0:64, :], lhsT=wq[:, c, :], rhs=xT[:, c, t0 : t0 + NQ],
                    start=(c == 0), stop=(c == NCH - 1),
                )
                nc.tensor.matmul(
                    qk[64:128, :], lhsT=wk[:, c, :], rhs=xT[:, c, t0 : t0 + NQ],
                    start=(c == 0), stop=(c == NCH - 1),
                )
            nc.vector.tensor_copy(out=qT2[0:64, t0 : t0 + NQ], in_=qk[0:64, :])
            nc.vector.tensor_copy(out=qT2[64:128, t0 : t0 + NQ], in_=qk[0:64, :])
            kv = qk[64:128, :].rearrange("d (x c k) -> d c x k", x=2, c=2, k=128)
            for par in range(2):
                nc.vector.tensor_copy(
                    out=kT2[par * 64 : par * 64 + 64, b,
                            (2 * qg) * 128 : (2 * qg) * 128 + 256],
                    in_=kv[:, par, :, :],
                )

        def emit_proj_v(tb):
            """v projection for one 128-token block"""
            vp = ps_sm.tile([128, DH], f32, tag="o", name="vp")
            for c in range(NCH):
                nc.tensor.matmul(
                    vp, lhsT=xT[:, c, tb * 128 : tb * 128 + 128], rhs=wv[:, c, :],
                    start=(c == 0), stop=(c == NCH - 1),
                )
            nc.vector.tensor_copy(out=vP[:, tb, 0:DH], in_=vp)

        def emit_proj_group(g):
            emit_proj_qk(g)
            for t in range(NQ // 128):
                emit_proj_v(g * (NQ // 128) + t)

        # proj groups 0-1 are the serial prefix; the rest become pieces
        # interleaved into the attention stream (each piece = one psum tile)
        emit_proj_group(0)
        emit_proj_group(1)
        pieces = []
        for g in range(2, NG):
            pieces.append(lambda g=g: emit_proj_qk(g))
            for t in range(NQ // 128):
                pieces.append(
                    lambda tb=g * (NQ // 128) + t: emit_proj_v(tb)
                )
        piece_i = 0

        # Phase 2+3: attention + output projection.
        #
        # oN holds the UNNORMALIZED attention output (bf16); the softmax
        # denominator (o row DH) is transposed into token-partition layout
        # via tiny K=1 matmuls, reciprocal'd wide, and applied as a
        # per-partition scalar fused into the projection's PSUM->SBUF copy.
        # Each group's epilogue (denominator transpose + projection) is
        # interleaved into the NEXT group's score loop so the in-order PE
        # queue never stalls on the normalize chain.
        def emit_denT(pq0, pden):
            denT = ps_sm.tile([128, 4], f32, tag="o", name="denT")
            for t in range(NQ // 128):
                nc.tensor.matmul(
                    denT[:, t : t + 1],
                    lhsT=pden[0:1, t * 128 : (t + 1) * 128],
                    rhs=ones1, start=True, stop=True,
                )
            recT = sb_io.tile([128, 4], f32, name="recT")
            nc.vector.reciprocal(recT, denT)
            return recT

        def emit_fp_one(pq0, t, recT):
            tt0 = pq0 + t * 128
            fp = ps_sm.tile([128, D], f32, tag="o", name="fp")
            nc.tensor.matmul(
                fp, lhsT=oN[:, tt0 : tt0 + 128], rhs=wo, start=True, stop=True
            )
            ob = sb_io.tile([128, D], f32, name="ob")
            nc.vector.tensor_scalar_mul(ob, in0=fp, scalar1=recT[:, t : t + 1])
            nc.sync.dma_start(out=out_d[tt0 : tt0 + 128, :], in_=ob)

        def emit_qk_score(s, b, blk, i, q0):
            # one score matmul for batch-local key block blk into s[:, i, :];
            # parity selects the PE row-half (row-tiled, K=64)
            par = blk % 2
            pr = blk >> 1
            nc.tensor.matmul(
                s[:, i, :],
                lhsT=kT2[par * 64 : par * 64 + 64, b, pr * 128 : pr * 128 + 128],
                rhs=qT2[par * 64 : par * 64 + 64, q0 : q0 + NQ],
                start=True, stop=True,
            )

        pending = None   # (q0, den tile) of the previous group
        pv_queue = None  # (o tile, b, p tile, jb, gsz) awaiting emission;
        #                  survives group boundaries so the last chunk's PV
        #                  overlaps the next group's first QK instead of
        #                  stalling on the final exp.

        def flush_pv(pv):
            po, pb, p, pjb, pgsz = pv
            for i in range(pgsz):
                jbg = pb * NJB + pjb + i
                nc.tensor.matmul(
                    po, lhsT=vP[:, jbg, :], rhs=p[:, i, :],
                    start=(pjb + i == 0), stop=(pjb + i == NJB - 1),
                )

        for gidx in range(NG):
            b, qg = gidx // (N // NQ), gidx % (N // NQ)
            q0 = b * N + qg * NQ
            o = None
            jb = 0
            recT = None
            # projection pieces per chunk: group 0 must front-run its own
            # key consumption (3/chunk); later groups drain the backlog
            nppc = 3 if gidx == 0 else 2
            for gi, gsz in enumerate(JGS):
                s = ps_s.tile([128, 3, NQ], f32, tag="s", name="s")
                # QK: one row-tiled even/odd pair + (maybe) one single.
                # The single is emitted LAST; on odd chunks it occupies PE
                # rows 64-127, so the previous group's epilogue matmuls
                # (fp: K=64 rows 0-63, denT: K=1 rows 0-31) emitted right
                # after it run concurrently with it.
                if jb % 2 == 0:
                    emit_qk_score(s, b, jb, 0, q0)
                    emit_qk_score(s, b, jb + 1, 1, q0)
                    if gsz == 3:
                        emit_qk_score(s, b, jb + 2, 2, q0)
                else:
                    emit_qk_score(s, b, jb + 1, 1, q0)
                    emit_qk_score(s, b, jb + 2, 2, q0)
                    emit_qk_score(s, b, jb, 0, q0)
                # previous group's fp projection, paired with the odd single
                if pending is not None and gi in (3, 5, 7, 9):
                    emit_fp_one(pending[0], (gi - 3) // 2, recT)
                if gi == 0:
                    # previous group's tail: flush its last PV chunk (overlaps
                    # this group's QK on the PE) and emit its oN/den copies
                    if pv_queue is not None:
                        flush_pv(pv_queue)
                        pv_queue = None
                        po, pq0 = pending_o
                        nc.vector.tensor_copy(
                            out=oN[:, pq0 : pq0 + NQ], in_=po[0:DH, :]
                        )
                        den = sb_io.tile([1, NQ], f32, name="den")
                        nc.vector.tensor_copy(out=den, in_=po[DH : DH + 1, :])
                        pending = (pq0, den)
                    o = ps_sm.tile([DH + 1, NQ], f32, tag="o", name="o")
                p = sb_p.tile([128, 3, NQ], bf16, name="p")
                # exp split: DVE (Schraudolph bit-trick) takes one EARLY
                # pair-member block (its scores land at chunk start, so the
                # 691ns DVE op finishes a full chunk before its PV consumer);
                # ScalarE takes the other two as a contiguous slice.  The
                # engines read different PSUM banks.
                if gsz == 3:
                    dvi = 0 if jb % 2 == 0 else 2  # early pair member slot
                    nc.vector.tensor_scalar(
                        out=p[:, dvi, :].bitcast(i16), in0=s[:, dvi, :],
                        scalar1=SCH_A, scalar2=SCH_B, op0=MULT, op1=ADD,
                    )
                    sc0 = 1 if dvi == 0 else 0
                    nc.scalar.activation(
                        out=p[:, sc0 : sc0 + 2, :], in_=s[:, sc0 : sc0 + 2, :],
                        func=EXP, scale=SCALE,
                    )
                else:
                    nc.scalar.activation(
                        out=p[:, 0:gsz, :], in_=s[:, 0:gsz, :], func=EXP,
                        scale=SCALE,
                    )
                # interleaved projection pieces (fill ScalarE-paced PE slack)
                for _pp in range(nppc):
                    if piece_i < len(pieces):
                        pieces[piece_i]()
                        piece_i += 1
                if pv_queue is not None:
                    flush_pv(pv_queue)
                pv_queue = (o, b, p, jb, gsz)
                jb += gsz
                # previous group's denominator transpose: after the PV flush
                # so its wait on the DVE den copy is covered by PE work
                if pending is not None and gi == 1:
                    recT = emit_denT(*pending)
            pending_o = (o, q0)

        flush_pv(pv_queue)
        po, pq0 = pending_o
        nc.vector.tensor_copy(out=oN[:, pq0 : pq0 + NQ], in_=po[0:DH, :])
        den = sb_io.tile([1, NQ], f32, name="den")
        nc.vector.tensor_copy(out=den, in_=po[DH : DH + 1, :])
        pending = (pq0, den)

        recT = emit_denT(*pending)
        for t in range(NQ // 128):
            emit_fp_one(pending[0], t, recT)

    nc.compile()
    return nc


def make_in_maps(x, Wq, Wk, Wv, Wo):
    bf16 = ml_dtypes.bfloat16
    # x transposed to [feat, tok] and chunked: [NCH, 128, TOK]
    x_bf = np.ascontiguousarray(
        x.reshape(TOK, D).T.reshape(NCH, 128, TOK)
    ).astype(bf16)
    in_maps = []
    for h in range(H):
        sl = slice(h * DH, (h + 1) * DH)
        in_maps.append(
            {
                "x": x_bf,
                "wq": np.ascontiguousarray(Wq[sl, :].T).astype(bf16),
                "wk": np.ascontiguousarray(Wk[sl, :].T).astype(bf16),
                "wv": np.ascontiguousarray(Wv[sl, :].T).astype(bf16),
                "wo": np.ascontiguousarray(Wo[:, sl].T).astype(bf16),
            }
        )
    return in_maps


def _install_ntff_shim():
    """The axon boot skips registering the NTFF profile hook when the image's
    antenv lacks axon_hooks; register an equivalent shim so trace=True works."""
    import types

    if "antenv.axon_hooks" in sys.modules:
        return
    try:
        from trn_agent_boot.trn_boot import _ntff_profile_via_ctypes

        hook = _ntff_profile_via_ctypes("/opt/axon/libaxon_pjrt.so")
    except Exception:
        hook = None
    mod = types.ModuleType("antenv.axon_hooks")
    mod.get_axon_ntff_profile_hook = lambda: hook
    sys.modules["antenv.axon_hooks"] = mod


def run(x, Wq, Wk, Wv, Wo, bo, trace=False):
    from concourse.bass_utils import run_bass_kernel_spmd

    if trace:
        _install_ntff_shim()

    nc = build_bass()
    in_maps = make_in_maps(x, Wq, Wk, Wv, Wo)
    res = run_bass_kernel_spmd(nc, in_maps, core_ids=list(range(H)), trace=trace)
    acc = np.zeros((TOK, D), dtype=np.float32)
    for r in res.results:
        acc += r["out"]
    acc += np.asarray(bo, dtype=np.float32)[None, :]
    return acc.reshape(B, N, D), res


def kernel(x, Wq, Wk, Wv, Wo, bo):
    out, _ = run(
        np.asarray(x, dtype=np.float32),
        np.asarray(Wq, dtype=np.float32),
        np.asarray(Wk, dtype=np.float32),
        np.asarray(Wv, dtype=np.float32),
        np.asarray(Wo, dtype=np.float32),
        np.asarray(bo, dtype=np.float32),
    )
    return out


# revision 18
# speedup vs baseline: 1.1415x; 1.1415x over previous
"""Trainium2 Bass kernel for nn_CrossAttention_82429012345074.

8-head self-attention, B=2, N=4096, d_model=512, 8 heads x 64 dim.

Sharding: one head per NeuronCore (8 heads / 8 cores) — tensor parallel:
to_q/k/v column-parallel (each core gets its head's 64 rows of Wq/Wk/Wv),
to_out row-parallel (each core gets its head's 64 columns of Wo and emits a
partial [tok, 512] output). The unshard step sums the 8 partials + bias on
host.

Per-core device kernel (all matmuls in bf16, fp32 accumulation):
  xT = dma(x pre-transposed on host)           # [512f, 8192t] in 4 chunks
  q/k proj: COL-TILED pair — wq into PE cols 0-63, wk into cols 64-127,
     both streaming the same xT chunk concurrently.  qk_ps[0:64]=q,
     [64:128]=k.  q is copied to BOTH partition halves of qT2; k is
     split even/odd key-block into kT2's partition halves.
  v     = xT.T @ Wv.T (natural layout)         # [8192, 64] + ones column
  per (batch, 512-query group), chunks of 3 key blocks:
     QK is ROW-TILED: the even/odd key blocks of a pair use PE rows
     0-63 / 64-127 (K=64 each) and run concurrently; the leftover
     single block uses whichever half its parity selects.
     pT = exp(sT * scale)                      # ScalarE, PSUM->SBUF bf16
     o[65, q] += [v|1].T @ pT                  # accumulate over j; row 64 = denom
     oN = o[0:64] (unnormalized, bf16); denom transposed via K=1 matmuls,
     reciprocal'd, applied as per-partition scalar in the projection's
     PSUM->SBUF copy.  Epilogue interleaved into the next group's chunks.

Pipelining: only projection groups 0-1 run before attention starts; the
remaining 14 projection groups (incl. all of batch 1) are interleaved into
the attention chunk stream as single-psum-tile "pieces" so their PE work
fills the slack of the ScalarE(exp)-paced steady state.  The kernel opens
with filler LDWEIGHTS so the PE HAM clock-gate is already warm when the
first real matmul issues.
"""

import sys

sys.path.insert(0, "/opt/trn_rl_repo")

import numpy as np
import ml_dtypes

B, N, D, H, DH = 2, 4096, 512, 8, 64
TOK = B * N            # 8192
NQ = 512               # query-group width
NCH = D // 128         # 4 feature chunks of x
NJB = N // 128         # 32 key blocks per batch
NPAIR = NJB // 2       # 16 key-block pairs per batch
NTB = TOK // 128       # 64 token blocks
NG = TOK // NQ         # 16 query groups (both batches)
JGS = [3] * 10 + [2]   # key-blocks per exp() chunk (sum = 32)
SCALE = DH ** -0.5
N_FILL = 0             # PE filler LDWEIGHTS per chunk (off: they stall the queue)
N_WARM = 48            # filler LDWEIGHTS at kernel start (HAM pre-warm)
# Schraudolph fast-exp constants (DVE path): bf16 bits of exp(s*SCALE) are
# approximated by int16(round(s * SCH_A + SCH_B)); the softmax denominator
# self-normalization cancels most of the sawtooth error (measured 2e-3).
SCH_A = SCALE * 128 * 1.4426950408889634
SCH_B = 16256.0 - 8.0


def build_bass():
    from contextlib import ExitStack

    import concourse.bass as bass
    import concourse.mybir as mybir
    import concourse.tile as tile
    from concourse import bacc

    f32 = mybir.dt.float32
    bf16 = mybir.dt.bfloat16
    i16 = mybir.dt.int16
    EXP = mybir.ActivationFunctionType.Exp
    MULT = mybir.AluOpType.mult
    ADD = mybir.AluOpType.add

    nc = bacc.Bacc("TRN2", target_bir_lowering=False, num_devices=8)
    x_d = nc.dram_tensor("x", [NCH, 128, TOK], bf16, kind="ExternalInput")
    wq_d = nc.dram_tensor("wq", [D, DH], bf16, kind="ExternalInput")
    wk_d = nc.dram_tensor("wk", [D, DH], bf16, kind="ExternalInput")
    wv_d = nc.dram_tensor("wv", [D, DH], bf16, kind="ExternalInput")
    wo_d = nc.dram_tensor("wo", [DH, D], bf16, kind="ExternalInput")
    out_d = nc.dram_tensor("out", [TOK, D], f32, kind="ExternalOutput")

    with tile.TileContext(nc) as tc, ExitStack() as ctx:
        const = ctx.enter_context(tc.tile_pool(name="const", bufs=1))
        sb_p = ctx.enter_context(tc.tile_pool(name="sb_p", bufs=3))
        sb_io = ctx.enter_context(tc.tile_pool(name="sb_io", bufs=3))
        ps_s = ctx.enter_context(tc.tile_pool(name="ps_s", bufs=2, space="PSUM"))
        ps_sm = ctx.enter_context(tc.tile_pool(name="ps_sm", bufs=2, space="PSUM"))

        # Long-lived SBUF tensors
        xT = const.tile([128, NCH, TOK], bf16, name="xT")      # x transposed, 4 chunks
        # qT duplicated in both partition halves (rhs for either QK row-tile)
        qT2 = const.tile([128, TOK], bf16, name="qT2")
        # kT split by key-block parity: partitions 0-63 = even blocks,
        # 64-127 = odd blocks; col = (block>>1)*128 + key
        kT2 = const.tile([128, B, NPAIR * 128], bf16, name="kT2")
        vP = const.tile([128, NTB, DH + 1], bf16, name="vP")   # v blocks + ones col
        oN = const.tile([64, TOK], bf16, name="oN")            # unnormalized attn out^T
        wq = const.tile([128, NCH, DH], bf16, name="wq")
        wk = const.tile([128, NCH, DH], bf16, name="wk")
        wv = const.tile([128, NCH, DH], bf16, name="wv")
        wo = const.tile([64, D], bf16, name="wo")
        warm = const.tile([128, 128], bf16, name="warm")

        nc.vector.memset(warm, 0.0)
        nc.vector.memset(vP[:, :, DH : DH + 1], 1.0)
        ones1 = const.tile([1, 1], f32, name="ones1")
        nc.vector.memset(ones1, 1.0)

        # HAM pre-warm: give the PE ~5us of LDWEIGHTS busy-work while the
        # x DMA lands, so the activity monitor un-gates the clock before the
        # first projection matmul.
        for _w in range(N_WARM):
            nc.tensor.ldweights(weights=warm)

        nc.sync.dma_start(out=wq, in_=wq_d[:].rearrange("(c p) d -> p c d", p=128))
        nc.sync.dma_start(out=wk, in_=wk_d[:].rearrange("(c p) d -> p c d", p=128))
        nc.sync.dma_start(out=wv, in_=wv_d[:].rearrange("(c p) d -> p c d", p=128))
        nc.sync.dma_start(out=wo, in_=wo_d[:])

        # Phase 0: x DMA, slab-major so early projection groups unblock first;
        # the first two slabs are 1024 cols so proj groups 0-1 start ASAP
        slabs = [(0, 1024), (1024, 2048)] + [
            (t0, t0 + 2048) for t0 in range(2048, TOK, 2048)
        ]
        for t0, t1 in slabs:
            for c in range(NCH):
                nc.sync.dma_start(
                    out=xT[:, c, t0:t1],
                    in_=x_d[c, :, t0:t1],
                )

        # ---- projection emitters ---------------------------------------
        def emit_proj_qk(g):
            """col-tiled q|k projection for query group g + layout copies"""
            b, qg, t0 = g // (N // NQ), g % (N // NQ), g * NQ
            qk = ps_sm.tile([128, NQ], f32, tag="o", name="qk")
            for c in range(NCH):
                nc.tensor.matmul(
                    qk[0:64, :], lhsT=wq[:, c, :], rhs=xT[:, c, t0 : t0 + NQ],
                    start=(c == 0), stop=(c == NCH - 1),
                )
                nc.tensor.matmul(
                    qk[64:128, :], lhsT=wk[:, c, :], rhs=xT[:, c, t0 : t0 + NQ],
                    start=(c == 0), stop=(c == NCH - 1),
                )
            nc.vector.tensor_copy(out=qT2[0:64, t0 : t0 + NQ], in_=qk[0:64, :])
            nc.vector.tensor_copy(out=qT2[64:128, t0 : t0 + NQ], in_=qk[0:64, :])
            kv = qk[64:128, :].rearrange("d (x c k) -> d c x k", x=2, c=2, k=128)
            for par in range(2):
                nc.vector.tensor_copy(
                    out=kT2[par * 64 : par * 64 + 64, b,
                            (2 * qg) * 128 : (2 * qg) * 128 + 256],
                    in_=kv[:, par, :, :],
                )

        def emit_proj_v(tb):
            """v projection for one 128-token block"""
            vp = ps_sm.tile([128, DH], f32, tag="o", name="vp")
            for c in range(NCH):
                nc.tensor.matmul(
                    vp, lhsT=xT[:, c, tb * 128 : tb * 128 + 128], rhs=wv[:, c, :],
                    start=(c == 0), stop=(c == NCH - 1),
                )
            nc.vector.tensor_copy(out=vP[:, tb, 0:DH], in_=vp)

        def emit_proj_group(g):
            emit_proj_qk(g)
            for t in range(NQ // 128):
                emit_proj_v(g * (NQ // 128) + t)

        # proj groups 0-1 are the serial prefix; the rest become pieces
        # interleaved into the attention stream (each piece = one psum tile)
        emit_proj_group(0)
        emit_proj_group(1)
        pieces = []
        for g in range(2, NG):
            pieces.append(lambda g=g: emit_proj_qk(g))
            for t in range(NQ // 128):
                pieces.append(
                    lambda tb=g * (NQ // 128) + t: emit_proj_v(tb)
                )
        piece_i = 0

        # Phase 2+3: attention + output projection.
        #
        # oN holds the UNNORMALIZED attention output (bf16); the softmax
        # denominator (o row DH) is transposed into token-partition layout
        # via tiny K=1 matmuls, reciprocal'd wide, and applied as a
        # per-partition scalar fused into the projection's PSUM->SBUF copy.
        # Each group's epilogue (denominator transpose + projection) is
        # interleaved into the NEXT group's score loop so the in-order PE
        # queue never stalls on the normalize chain.
        def emit_denT(pq0, pden):
            denT = ps_sm.tile([128, 4], f32, tag="o", name="denT")
            for t in range(NQ // 128):
                nc.tensor.matmul(
                    denT[:, t : t + 1],
                    lhsT=pden[0:1, t * 128 : (t + 1) * 128],
                    rhs=ones1, start=True, stop=True,
                )
            recT = sb_io.tile([128, 4], f32, name="recT")
            nc.vector.reciprocal(recT, denT)
            return recT

        def emit_fp_one(pq0, t, recT):
            tt0 = pq0 + t * 128
            fp = ps_sm.tile([128, D], f32, tag="o", name="fp")
            nc.tensor.matmul(
                fp, lhsT=oN[:, tt0 : tt0 + 128], rhs=wo, start=True, stop=True
            )
            ob = sb_io.tile([128, D], f32, name="ob")
            nc.vector.tensor_scalar_mul(ob, in0=fp, scalar1=recT[:, t : t + 1])
            nc.sync.dma_start(out=out_d[tt0 : tt0 + 128, :], in_=ob)

        def emit_qk_score(s, b, blk, i, q0):
            # one score matmul for batch-local key block blk into s[:, i, :];
            # parity selects the PE row-half (row-tiled, K=64)
            par = blk % 2
            pr = blk >> 1
            nc.tensor.matmul(
                s[:, i, :],
                lhsT=kT2[par * 64 : par * 64 + 64, b, pr * 128 : pr * 128 + 128],
                rhs=qT2[par * 64 : par * 64 + 64, q0 : q0 + NQ],
                start=True, stop=True,
            )

        pending = None   # (q0, den tile) of the previous group
        pv_queue = None  # (o tile, b, p tile, jb, gsz) awaiting emission;
        #                  survives group boundaries so the last chunk's PV
        #                  overlaps the next group's first QK instead of
        #                  stalling on the final exp.

        def flush_pv(pv):
            po, pb, p, pjb, pgsz = pv
            for i in range(pgsz):
                jbg = pb * NJB + pjb + i
                nc.tensor.matmul(
                    po, lhsT=vP[:, jbg, :], rhs=p[:, i, :],
                    start=(pjb + i == 0), stop=(pjb + i == NJB - 1),
                )

        for gidx in range(NG):
            b, qg = gidx // (N // NQ), gidx % (N // NQ)
            q0 = b * N + qg * NQ
            o = None
            jb = 0
            recT = None
            # projection pieces per chunk: group 0 must front-run its own
            # key consumption (3/chunk); later groups drain the backlog
            nppc = 3 if gidx == 0 else 2
            for gi, gsz in enumerate(JGS):
                s = ps_s.tile([128, 3, NQ], f32, tag="s", name="s")
                # QK: one row-tiled even/odd pair + (maybe) one single.
                # The single is emitted LAST; on odd chunks it occupies PE
                # rows 64-127, so the previous group's epilogue matmuls
                # (fp: K=64 rows 0-63, denT: K=1 rows 0-31) emitted right
                # after it run concurrently with it.
                if jb % 2 == 0:
                    emit_qk_score(s, b, jb, 0, q0)
                    emit_qk_score(s, b, jb + 1, 1, q0)
                    if gsz == 3:
                        emit_qk_score(s, b, jb + 2, 2, q0)
                else:
                    emit_qk_score(s, b, jb + 1, 1, q0)
                    emit_qk_score(s, b, jb + 2, 2, q0)
                    emit_qk_score(s, b, jb, 0, q0)
                # previous group's fp projection, paired with the odd single
                if pending is not None and gi in (3, 5, 7, 9):
                    emit_fp_one(pending[0], (gi - 3) // 2, recT)
                if gi == 0:
                    # previous group's tail: flush its last PV chunk (overlaps
                    # this group's QK on the PE) and emit its oN/den copies
                    if pv_queue is not None:
                        flush_pv(pv_queue)
                        pv_queue = None
                        po, pq0 = pending_o
                        nc.vector.tensor_copy(
                            out=oN[:, pq0 : pq0 + NQ], in_=po[0:DH, :]
                        )
                        den = sb_io.tile([1, NQ], f32, name="den")
                        nc.vector.tensor_copy(out=den, in_=po[DH : DH + 1, :])
                        pending = (pq0, den)
                    o = ps_sm.tile([DH + 1, NQ], f32, tag="o", name="o")
                p = sb_p.tile([128, 3, NQ], bf16, name="p")
                # exp split: DVE (Schraudolph bit-trick) takes one EARLY
                # pair-member block (its scores land at chunk start, so the
                # 691ns DVE op finishes a full chunk before its PV consumer);
                # ScalarE takes the other two as a contiguous slice.  The
                # engines read different PSUM banks.
                if gsz == 3:
                    # ScalarE exps the PAIR (scores ready at chunk start, so
                    # the ACT starts immediately and the next chunk's PV
                    # never stalls on it); DVE Schraudolphs the SINGLE.
                    dvi = 2 if jb % 2 == 0 else 0  # the single's slot
                    sc0 = 0 if dvi == 2 else 1
                    nc.scalar.activation(
                        out=p[:, sc0 : sc0 + 2, :], in_=s[:, sc0 : sc0 + 2, :],
                        func=EXP, scale=SCALE,
                    )
                    nc.vector.tensor_scalar(
                        out=p[:, dvi, :].bitcast(i16), in0=s[:, dvi, :],
                        scalar1=SCH_A, scalar2=SCH_B, op0=MULT, op1=ADD,
                    )
                else:
                    nc.scalar.activation(
                        out=p[:, 0:gsz, :], in_=s[:, 0:gsz, :], func=EXP,
                        scale=SCALE,
                    )
                # interleaved projection pieces (fill ScalarE-paced PE slack)
                for _pp in range(nppc):
                    if piece_i < len(pieces):
                        pieces[piece_i]()
                        piece_i += 1
                if pv_queue is not None:
                    flush_pv(pv_queue)
                pv_queue = (o, b, p, jb, gsz)
                jb += gsz
                # previous group's denominator transpose: after the PV flush
                # so its wait on the DVE den copy is covered by PE work
                if pending is not None and gi == 1:
                    recT = emit_denT(*pending)
            pending_o = (o, q0)

        flush_pv(pv_queue)
        po, pq0 = pending_o
        nc.vector.tensor_copy(out=oN[:, pq0 : pq0 + NQ], in_=po[0:DH, :])
        den = sb_io.tile([1, NQ], f32, name="den")
        nc.vector.tensor_copy(out=den, in_=po[DH : DH + 1, :])
        pending = (pq0, den)

        recT = emit_denT(*pending)
        for t in range(NQ // 128):
            emit_fp_one(pending[0], t, recT)

    nc.compile()
    return nc


def make_in_maps(x, Wq, Wk, Wv, Wo):
    bf16 = ml_dtypes.bfloat16
    # x transposed to [feat, tok] and chunked: [NCH, 128, TOK]
    x_bf = np.ascontiguousarray(
        x.reshape(TOK, D).T.reshape(NCH, 128, TOK)
    ).astype(bf16)
    in_maps = []
    for h in range(H):
        sl = slice(h * DH, (h + 1) * DH)
        in_maps.append(
            {
                "x": x_bf,
                "wq": np.ascontiguousarray(Wq[sl, :].T).astype(bf16),
                "wk": np.ascontiguousarray(Wk[sl, :].T).astype(bf16),
                "wv": np.ascontiguousarray(Wv[sl, :].T).astype(bf16),
                "wo": np.ascontiguousarray(Wo[:, sl].T).astype(bf16),
            }
        )
    return in_maps


def _install_ntff_shim():
    """The axon boot skips registering the NTFF profile hook when the image's
    antenv lacks axon_hooks; register an equivalent shim so trace=True works."""
    import types

    if "antenv.axon_hooks" in sys.modules:
        return
    try:
        from trn_agent_boot.trn_boot import _ntff_profile_via_ctypes

        hook = _ntff_profile_via_ctypes("/opt/axon/libaxon_pjrt.so")
    except Exception:
        hook = None
    mod = types.ModuleType("antenv.axon_hooks")
    mod.get_axon_ntff_profile_hook = lambda: hook
    sys.modules["antenv.axon_hooks"] = mod


def run(x, Wq, Wk, Wv, Wo, bo, trace=False):
    from concourse.bass_utils import run_bass_kernel_spmd

    if trace:
        _install_ntff_shim()

    nc = build_bass()
    in_maps = make_in_maps(x, Wq, Wk, Wv, Wo)
    res = run_bass_kernel_spmd(nc, in_maps, core_ids=list(range(H)), trace=trace)
    acc = np.zeros((TOK, D), dtype=np.float32)
    for r in res.results:
        acc += r["out"]
    acc += np.asarray(bo, dtype=np.float32)[None, :]
    return acc.reshape(B, N, D), res


def kernel(x, Wq, Wk, Wv, Wo, bo):
    out, _ = run(
        np.asarray(x, dtype=np.float32),
        np.asarray(Wq, dtype=np.float32),
        np.asarray(Wk, dtype=np.float32),
        np.asarray(Wv, dtype=np.float32),
        np.asarray(Wo, dtype=np.float32),
        np.asarray(bo, dtype=np.float32),
    )
    return out


# revision 20
# speedup vs baseline: 1.1670x; 1.0223x over previous
"""Trainium2 Bass kernel for nn_CrossAttention_82429012345074.

8-head self-attention, B=2, N=4096, d_model=512, 8 heads x 64 dim.

Sharding: one head per NeuronCore (8 heads / 8 cores) — tensor parallel:
to_q/k/v column-parallel (each core gets its head's 64 rows of Wq/Wk/Wv),
to_out row-parallel (each core gets its head's 64 columns of Wo and emits a
partial [tok, 512] output). The unshard step sums the 8 partials + bias on
host.

Per-core device kernel (all matmuls in bf16, fp32 accumulation):
  xT = dma(x pre-transposed on host)           # [512f, 8192t] in 4 chunks
  q/k proj: COL-TILED pair — wq into PE cols 0-63, wk into cols 64-127,
     both streaming the same xT chunk concurrently.  qk_ps[0:64]=q,
     [64:128]=k.  q is copied to BOTH partition halves of qT2; k is
     split even/odd key-block into kT2's partition halves.
  v     = xT.T @ Wv.T (natural layout)         # [8192, 64] + ones column
  per (batch, 512-query group), chunks of 3 key blocks:
     QK is ROW-TILED: the even/odd key blocks of a pair use PE rows
     0-63 / 64-127 (K=64 each) and run concurrently; the leftover
     single block uses whichever half its parity selects.
     pT = exp(sT * scale)                      # ScalarE, PSUM->SBUF bf16
     o[65, q] += [v|1].T @ pT                  # accumulate over j; row 64 = denom
     oN = o[0:64] (unnormalized, bf16); denom transposed via K=1 matmuls,
     reciprocal'd, applied as per-partition scalar in the projection's
     PSUM->SBUF copy.  Epilogue interleaved into the next group's chunks.

Pipelining: only projection groups 0-1 run before attention starts; the
remaining 14 projection groups (incl. all of batch 1) are interleaved into
the attention chunk stream as single-psum-tile "pieces" so their PE work
fills the slack of the ScalarE(exp)-paced steady state.  The kernel opens
with filler LDWEIGHTS so the PE HAM clock-gate is already warm when the
first real matmul issues.
"""

import sys

sys.path.insert(0, "/opt/trn_rl_repo")

import numpy as np
import ml_dtypes

B, N, D, H, DH = 2, 4096, 512, 8, 64
TOK = B * N            # 8192
NQ = 512               # query-group width
NCH = D // 128         # 4 feature chunks of x
NJB = N // 128         # 32 key blocks per batch
NPAIR = NJB // 2       # 16 key-block pairs per batch
NTB = TOK // 128       # 64 token blocks
NG = TOK // NQ         # 16 query groups (both batches)
JGS = [3] * 10 + [2]   # key-blocks per exp() chunk (sum = 32)
SCALE = DH ** -0.5
N_FILL = 0             # PE filler LDWEIGHTS per chunk (off: they stall the queue)
N_WARM = 48            # filler LDWEIGHTS at kernel start (HAM pre-warm)
# Schraudolph fast-exp constants (DVE path): bf16 bits of exp(s*SCALE) are
# approximated by int16(round(s * SCH_A + SCH_B)); the softmax denominator
# self-normalization cancels most of the sawtooth error (measured 2e-3).
SCH_A = SCALE * 128 * 1.4426950408889634
SCH_B = 16256.0 - 8.0


def build_bass():
    from contextlib import ExitStack

    import concourse.bass as bass
    import concourse.mybir as mybir
    import concourse.tile as tile
    from concourse import bacc

    f32 = mybir.dt.float32
    bf16 = mybir.dt.bfloat16
    i16 = mybir.dt.int16
    EXP = mybir.ActivationFunctionType.Exp
    MULT = mybir.AluOpType.mult
    ADD = mybir.AluOpType.add

    nc = bacc.Bacc("TRN2", target_bir_lowering=False, num_devices=8)
    x_d = nc.dram_tensor("x", [NCH, 128, TOK], bf16, kind="ExternalInput")
    wq_d = nc.dram_tensor("wq", [D, DH], bf16, kind="ExternalInput")
    wk_d = nc.dram_tensor("wk", [D, DH], bf16, kind="ExternalInput")
    wv_d = nc.dram_tensor("wv", [D, DH], bf16, kind="ExternalInput")
    wo_d = nc.dram_tensor("wo", [DH, D], bf16, kind="ExternalInput")
    out_d = nc.dram_tensor("out", [TOK, D], f32, kind="ExternalOutput")

    with tile.TileContext(nc) as tc, ExitStack() as ctx:
        const = ctx.enter_context(tc.tile_pool(name="const", bufs=1))
        sb_p = ctx.enter_context(tc.tile_pool(name="sb_p", bufs=3))
        sb_io = ctx.enter_context(tc.tile_pool(name="sb_io", bufs=3))
        ps_s = ctx.enter_context(tc.tile_pool(name="ps_s", bufs=2, space="PSUM"))
        ps_sm = ctx.enter_context(tc.tile_pool(name="ps_sm", bufs=2, space="PSUM"))

        # Long-lived SBUF tensors
        xT = const.tile([128, NCH, TOK], bf16, name="xT")      # x transposed, 4 chunks
        # qT duplicated in both partition halves (rhs for either QK row-tile)
        qT2 = const.tile([128, TOK], bf16, name="qT2")
        # kT split by key-block parity: partitions 0-63 = even blocks,
        # 64-127 = odd blocks; col = (block>>1)*128 + key
        kT2 = const.tile([128, B, NPAIR * 128], bf16, name="kT2")
        vP = const.tile([128, NTB, DH + 1], bf16, name="vP")   # v blocks + ones col
        oN = const.tile([64, TOK], bf16, name="oN")            # unnormalized attn out^T
        wq = const.tile([128, NCH, DH], bf16, name="wq")
        wk = const.tile([128, NCH, DH], bf16, name="wk")
        wv = const.tile([128, NCH, DH], bf16, name="wv")
        wo = const.tile([64, D], bf16, name="wo")
        warm = const.tile([128, 128], bf16, name="warm")

        nc.vector.memset(warm, 0.0)
        nc.vector.memset(vP[:, :, DH : DH + 1], 1.0)
        ones1 = const.tile([1, 1], f32, name="ones1")
        nc.vector.memset(ones1, 1.0)

        # HAM pre-warm: give the PE ~5us of LDWEIGHTS busy-work while the
        # x DMA lands, so the activity monitor un-gates the clock before the
        # first projection matmul.
        for _w in range(N_WARM):
            nc.tensor.ldweights(weights=warm)

        nc.sync.dma_start(out=wq, in_=wq_d[:].rearrange("(c p) d -> p c d", p=128))
        nc.sync.dma_start(out=wk, in_=wk_d[:].rearrange("(c p) d -> p c d", p=128))
        nc.sync.dma_start(out=wv, in_=wv_d[:].rearrange("(c p) d -> p c d", p=128))
        nc.sync.dma_start(out=wo, in_=wo_d[:])

        # Phase 0: x DMA, slab-major so early projection groups unblock first;
        # the first two slabs are 1024 cols so proj groups 0-1 start ASAP
        slabs = [(0, 1024), (1024, 2048)] + [
            (t0, t0 + 2048) for t0 in range(2048, TOK, 2048)
        ]
        for t0, t1 in slabs:
            for c in range(NCH):
                nc.sync.dma_start(
                    out=xT[:, c, t0:t1],
                    in_=x_d[c, :, t0:t1],
                )

        # ---- projection emitters ---------------------------------------
        def emit_proj_qk(g):
            """col-tiled q|k projection for query group g + layout copies"""
            b, qg, t0 = g // (N // NQ), g % (N // NQ), g * NQ
            qk = ps_sm.tile([128, NQ], f32, tag="o", name="qk")
            for c in range(NCH):
                nc.tensor.matmul(
                    qk[0:64, :], lhsT=wq[:, c, :], rhs=xT[:, c, t0 : t0 + NQ],
                    start=(c == 0), stop=(c == NCH - 1),
                )
                nc.tensor.matmul(
                    qk[64:128, :], lhsT=wk[:, c, :], rhs=xT[:, c, t0 : t0 + NQ],
                    start=(c == 0), stop=(c == NCH - 1),
                )
            nc.vector.tensor_copy(out=qT2[0:64, t0 : t0 + NQ], in_=qk[0:64, :])
            nc.vector.tensor_copy(out=qT2[64:128, t0 : t0 + NQ], in_=qk[0:64, :])
            kv = qk[64:128, :].rearrange("d (x c k) -> d c x k", x=2, c=2, k=128)
            for par in range(2):
                nc.vector.tensor_copy(
                    out=kT2[par * 64 : par * 64 + 64, b,
                            (2 * qg) * 128 : (2 * qg) * 128 + 256],
                    in_=kv[:, par, :, :],
                )

        def emit_proj_v(tb):
            """v projection for one 128-token block"""
            vp = ps_sm.tile([128, DH], f32, tag="o", name="vp")
            for c in range(NCH):
                nc.tensor.matmul(
                    vp, lhsT=xT[:, c, tb * 128 : tb * 128 + 128], rhs=wv[:, c, :],
                    start=(c == 0), stop=(c == NCH - 1),
                )
            nc.vector.tensor_copy(out=vP[:, tb, 0:DH], in_=vp)

        def emit_proj_group(g):
            emit_proj_qk(g)
            for t in range(NQ // 128):
                emit_proj_v(g * (NQ // 128) + t)

        # proj groups 0-1 are the serial prefix; the rest become pieces
        # interleaved into the attention stream (each piece = one psum tile)
        emit_proj_group(0)
        emit_proj_group(1)
        pieces = []
        for g in range(2, NG):
            pieces.append(lambda g=g: emit_proj_qk(g))
            for t in range(NQ // 128):
                pieces.append(
                    lambda tb=g * (NQ // 128) + t: emit_proj_v(tb)
                )
        piece_i = 0

        # Phase 2+3: attention + output projection.
        #
        # oN holds the UNNORMALIZED attention output (bf16); the softmax
        # denominator (o row DH) is transposed into token-partition layout
        # via tiny K=1 matmuls, reciprocal'd wide, and applied as a
        # per-partition scalar fused into the projection's PSUM->SBUF copy.
        # Each group's epilogue (denominator transpose + projection) is
        # interleaved into the NEXT group's score loop so the in-order PE
        # queue never stalls on the normalize chain.
        def emit_denT(pq0, pden):
            denT = ps_sm.tile([128, 4], f32, tag="o", name="denT")
            for t in range(NQ // 128):
                nc.tensor.matmul(
                    denT[:, t : t + 1],
                    lhsT=pden[0:1, t * 128 : (t + 1) * 128],
                    rhs=ones1, start=True, stop=True,
                )
            recT = sb_io.tile([128, 4], f32, name="recT")
            nc.vector.reciprocal(recT, denT)
            return recT

        def emit_fp_one(pq0, t, recT):
            tt0 = pq0 + t * 128
            fp = ps_sm.tile([128, D], f32, tag="o", name="fp")
            nc.tensor.matmul(
                fp, lhsT=oN[:, tt0 : tt0 + 128], rhs=wo, start=True, stop=True
            )
            ob = sb_io.tile([128, D], f32, name="ob")
            nc.vector.tensor_scalar_mul(ob, in0=fp, scalar1=recT[:, t : t + 1])
            nc.sync.dma_start(out=out_d[tt0 : tt0 + 128, :], in_=ob)

        def emit_qk_score(s, b, blk, i, q0):
            # one score matmul for batch-local key block blk into s[:, i, :];
            # parity selects the PE row-half (row-tiled, K=64)
            par = blk % 2
            pr = blk >> 1
            nc.tensor.matmul(
                s[:, i, :],
                lhsT=kT2[par * 64 : par * 64 + 64, b, pr * 128 : pr * 128 + 128],
                rhs=qT2[par * 64 : par * 64 + 64, q0 : q0 + NQ],
                start=True, stop=True,
            )

        pending = None   # (q0, den tile) of the previous group
        pv_queue = None  # (o tile, b, p tile, jb, gsz) awaiting emission;
        #                  survives group boundaries so the last chunk's PV
        #                  overlaps the next group's first QK instead of
        #                  stalling on the final exp.

        def flush_pv(pv):
            po, pb, p, pjb, pgsz = pv
            for i in range(pgsz):
                jbg = pb * NJB + pjb + i
                nc.tensor.matmul(
                    po, lhsT=vP[:, jbg, :], rhs=p[:, i, :],
                    start=(pjb + i == 0), stop=(pjb + i == NJB - 1),
                )

        for gidx in range(NG):
            b, qg = gidx // (N // NQ), gidx % (N // NQ)
            q0 = b * N + qg * NQ
            o = None
            jb = 0
            recT = None
            # projection pieces per chunk: group 0 must front-run its own
            # key consumption (3/chunk); later groups drain the backlog
            nppc = 3 if gidx == 0 else 2
            for gi, gsz in enumerate(JGS):
                s = ps_s.tile([128, 3, NQ], f32, tag="s", name="s")
                # QK: one row-tiled even/odd pair + (maybe) one single.
                # The single is emitted LAST; on odd chunks it occupies PE
                # rows 64-127, so the previous group's epilogue matmuls
                # (fp: K=64 rows 0-63, denT: K=1 rows 0-31) emitted right
                # after it run concurrently with it.
                if jb % 2 == 0:
                    emit_qk_score(s, b, jb, 0, q0)
                    emit_qk_score(s, b, jb + 1, 1, q0)
                    if gsz == 3:
                        emit_qk_score(s, b, jb + 2, 2, q0)
                else:
                    emit_qk_score(s, b, jb + 1, 1, q0)
                    emit_qk_score(s, b, jb + 2, 2, q0)
                    emit_qk_score(s, b, jb, 0, q0)
                # previous group's fp projection, paired with the odd single
                if pending is not None and gi in (3, 5, 7, 9):
                    emit_fp_one(pending[0], (gi - 3) // 2, recT)
                if gi == 0:
                    # previous group's tail: flush its last PV chunk (overlaps
                    # this group's QK on the PE) and emit its oN/den copies
                    if pv_queue is not None:
                        flush_pv(pv_queue)
                        pv_queue = None
                        po, pq0 = pending_o
                        nc.vector.tensor_copy(
                            out=oN[:, pq0 : pq0 + NQ], in_=po[0:DH, :]
                        )
                        den = sb_io.tile([1, NQ], f32, name="den")
                        nc.vector.tensor_copy(out=den, in_=po[DH : DH + 1, :])
                        pending = (pq0, den)
                    o = ps_sm.tile([DH + 1, NQ], f32, tag="o", name="o")
                p = sb_p.tile([128, 3, NQ], bf16, name="p")
                # exp split: DVE (Schraudolph bit-trick) takes one EARLY
                # pair-member block (its scores land at chunk start, so the
                # 691ns DVE op finishes a full chunk before its PV consumer);
                # ScalarE takes the other two as a contiguous slice.  The
                # engines read different PSUM banks.
                nsc = 2 if gsz == 3 else gsz
                nc.scalar.activation(
                    out=p[:, 0:nsc, :], in_=s[:, 0:nsc, :], func=EXP, scale=SCALE
                )
                # interleaved projection pieces (fill ScalarE-paced PE slack)
                for _pp in range(nppc):
                    if piece_i < len(pieces):
                        pieces[piece_i]()
                        piece_i += 1
                if pv_queue is not None:
                    flush_pv(pv_queue)
                # DVE (Schraudolph bit-trick) exps block 2.  Emitted AFTER
                # the PV flush: the bitcast makes this an imprecise writer of
                # the p pool, so any PV matmul emitted after it waits for it
                # conservatively — after the flush, that ordering coincides
                # with the true dependency (only the NEXT flush needs it).
                if gsz == 3:
                    nc.vector.tensor_scalar(
                        out=p[:, 2, :].bitcast(i16), in0=s[:, 2, :],
                        scalar1=SCH_A, scalar2=SCH_B, op0=MULT, op1=ADD,
                    )
                pv_queue = (o, b, p, jb, gsz)
                jb += gsz
                # previous group's denominator transpose: after the PV flush
                # so its wait on the DVE den copy is covered by PE work
                if pending is not None and gi == 1:
                    recT = emit_denT(*pending)
            pending_o = (o, q0)

        flush_pv(pv_queue)
        po, pq0 = pending_o
        nc.vector.tensor_copy(out=oN[:, pq0 : pq0 + NQ], in_=po[0:DH, :])
        den = sb_io.tile([1, NQ], f32, name="den")
        nc.vector.tensor_copy(out=den, in_=po[DH : DH + 1, :])
        pending = (pq0, den)

        recT = emit_denT(*pending)
        for t in range(NQ // 128):
            emit_fp_one(pending[0], t, recT)

    nc.compile()
    return nc


def make_in_maps(x, Wq, Wk, Wv, Wo):
    bf16 = ml_dtypes.bfloat16
    # x transposed to [feat, tok] and chunked: [NCH, 128, TOK]
    x_bf = np.ascontiguousarray(
        x.reshape(TOK, D).T.reshape(NCH, 128, TOK)
    ).astype(bf16)
    in_maps = []
    for h in range(H):
        sl = slice(h * DH, (h + 1) * DH)
        in_maps.append(
            {
                "x": x_bf,
                "wq": np.ascontiguousarray(Wq[sl, :].T).astype(bf16),
                "wk": np.ascontiguousarray(Wk[sl, :].T).astype(bf16),
                "wv": np.ascontiguousarray(Wv[sl, :].T).astype(bf16),
                "wo": np.ascontiguousarray(Wo[:, sl].T).astype(bf16),
            }
        )
    return in_maps


def _install_ntff_shim():
    """The axon boot skips registering the NTFF profile hook when the image's
    antenv lacks axon_hooks; register an equivalent shim so trace=True works."""
    import types

    if "antenv.axon_hooks" in sys.modules:
        return
    try:
        from trn_agent_boot.trn_boot import _ntff_profile_via_ctypes

        hook = _ntff_profile_via_ctypes("/opt/axon/libaxon_pjrt.so")
    except Exception:
        hook = None
    mod = types.ModuleType("antenv.axon_hooks")
    mod.get_axon_ntff_profile_hook = lambda: hook
    sys.modules["antenv.axon_hooks"] = mod


def run(x, Wq, Wk, Wv, Wo, bo, trace=False):
    from concourse.bass_utils import run_bass_kernel_spmd

    if trace:
        _install_ntff_shim()

    nc = build_bass()
    in_maps = make_in_maps(x, Wq, Wk, Wv, Wo)
    res = run_bass_kernel_spmd(nc, in_maps, core_ids=list(range(H)), trace=trace)
    acc = np.zeros((TOK, D), dtype=np.float32)
    for r in res.results:
        acc += r["out"]
    acc += np.asarray(bo, dtype=np.float32)[None, :]
    return acc.reshape(B, N, D), res


def kernel(x, Wq, Wk, Wv, Wo, bo):
    out, _ = run(
        np.asarray(x, dtype=np.float32),
        np.asarray(Wq, dtype=np.float32),
        np.asarray(Wk, dtype=np.float32),
        np.asarray(Wv, dtype=np.float32),
        np.asarray(Wo, dtype=np.float32),
        np.asarray(bo, dtype=np.float32),
    )
    return out


# revision 25
# speedup vs baseline: 1.1805x; 1.0116x over previous
"""Trainium2 Bass kernel for nn_CrossAttention_82429012345074.

8-head self-attention, B=2, N=4096, d_model=512, 8 heads x 64 dim.

Sharding: one head per NeuronCore (8 heads / 8 cores) — tensor parallel:
to_q/k/v column-parallel (each core gets its head's 64 rows of Wq/Wk/Wv),
to_out row-parallel (each core gets its head's 64 columns of Wo and emits a
partial [tok, 512] output). The unshard step sums the 8 partials + bias on
host.

Per-core device kernel (all matmuls in bf16, fp32 accumulation):
  xT = dma(x pre-transposed on host)           # [512f, 8192t] in 4 chunks
  q/k proj: COL-TILED pair — wq into PE cols 0-63, wk into cols 64-127,
     both streaming the same xT chunk concurrently.  qk_ps[0:64]=q,
     [64:128]=k.  q is copied to BOTH partition halves of qT2; k is
     split even/odd key-block into kT2's partition halves.
  v     = xT.T @ Wv.T (natural layout)         # [8192, 64] + ones column
  per (batch, 512-query group), chunks of 3 key blocks:
     QK is ROW-TILED: the even/odd key blocks of a pair use PE rows
     0-63 / 64-127 (K=64 each) and run concurrently; the leftover
     single block uses whichever half its parity selects.
     pT = exp(sT * scale)                      # ScalarE, PSUM->SBUF bf16
     o[65, q] += [v|1].T @ pT                  # accumulate over j; row 64 = denom
     oN = o[0:64] (unnormalized, bf16); denom transposed via K=1 matmuls,
     reciprocal'd, applied as per-partition scalar in the projection's
     PSUM->SBUF copy.  Epilogue interleaved into the next group's chunks.

Pipelining: only projection groups 0-1 run before attention starts; the
remaining 14 projection groups (incl. all of batch 1) are interleaved into
the attention chunk stream as single-psum-tile "pieces" so their PE work
fills the slack of the ScalarE(exp)-paced steady state.  The kernel opens
with filler LDWEIGHTS so the PE HAM clock-gate is already warm when the
first real matmul issues.
"""

import sys

sys.path.insert(0, "/opt/trn_rl_repo")

import numpy as np
import ml_dtypes

B, N, D, H, DH = 2, 4096, 512, 8, 64
TOK = B * N            # 8192
NQ = 512               # query-group width
NCH = D // 128         # 4 feature chunks of x
NJB = N // 128         # 32 key blocks per batch
NPAIR = NJB // 2       # 16 key-block pairs per batch
NTB = TOK // 128       # 64 token blocks
NG = TOK // NQ         # 16 query groups (both batches)
JGS = [3] * 10 + [2]   # key-blocks per exp() chunk (sum = 32)
SCALE = DH ** -0.5
N_FILL = 0             # PE filler LDWEIGHTS per chunk (off: they stall the queue)
N_WARM = 48            # filler LDWEIGHTS at kernel start (HAM pre-warm)
# Schraudolph fast-exp constants (DVE path): bf16 bits of exp(s*SCALE) are
# approximated by int16(round(s * SCH_A + SCH_B)); the softmax denominator
# self-normalization cancels most of the sawtooth error (measured 2e-3).
SCH_A = SCALE * 128 * 1.4426950408889634
SCH_B = 16256.0 - 8.0


def build_bass():
    from contextlib import ExitStack

    import concourse.bass as bass
    import concourse.mybir as mybir
    import concourse.tile as tile
    from concourse import bacc

    f32 = mybir.dt.float32
    bf16 = mybir.dt.bfloat16
    i16 = mybir.dt.int16
    EXP = mybir.ActivationFunctionType.Exp
    MULT = mybir.AluOpType.mult
    ADD = mybir.AluOpType.add

    nc = bacc.Bacc("TRN2", target_bir_lowering=False, num_devices=8)
    x_d = nc.dram_tensor("x", [NCH, 128, TOK], bf16, kind="ExternalInput")
    wq_d = nc.dram_tensor("wq", [D, DH], bf16, kind="ExternalInput")
    wk_d = nc.dram_tensor("wk", [D, DH], bf16, kind="ExternalInput")
    wv_d = nc.dram_tensor("wv", [D, DH], bf16, kind="ExternalInput")
    wo_d = nc.dram_tensor("wo", [DH, D], bf16, kind="ExternalInput")
    out_d = nc.dram_tensor("out", [TOK, D], f32, kind="ExternalOutput")

    with tile.TileContext(nc) as tc, ExitStack() as ctx:
        const = ctx.enter_context(tc.tile_pool(name="const", bufs=1))
        sb_p = ctx.enter_context(tc.tile_pool(name="sb_p", bufs=3))
        sb_io = ctx.enter_context(tc.tile_pool(name="sb_io", bufs=3))
        ps_s = ctx.enter_context(tc.tile_pool(name="ps_s", bufs=2, space="PSUM"))
        ps_sm = ctx.enter_context(tc.tile_pool(name="ps_sm", bufs=2, space="PSUM"))

        # Long-lived SBUF tensors
        xT = const.tile([128, NCH, TOK], bf16, name="xT")      # x transposed, 4 chunks
        # qT duplicated in both partition halves (rhs for either QK row-tile)
        qT2 = const.tile([128, TOK], bf16, name="qT2")
        # kT split by key-block parity: partitions 0-63 = even blocks,
        # 64-127 = odd blocks; col = (block>>1)*128 + key
        kT2 = const.tile([128, B, NPAIR * 128], bf16, name="kT2")
        vP = const.tile([128, NTB, DH + 1], bf16, name="vP")   # v blocks + ones col
        oN = const.tile([64, TOK], bf16, name="oN")            # unnormalized attn out^T
        wq = const.tile([128, NCH, DH], bf16, name="wq")
        wk = const.tile([128, NCH, DH], bf16, name="wk")
        wv = const.tile([128, NCH, DH], bf16, name="wv")
        wo = const.tile([64, D], bf16, name="wo")
        warm = const.tile([128, 128], bf16, name="warm")

        nc.vector.memset(warm, 0.0)
        nc.vector.memset(vP[:, :, DH : DH + 1], 1.0)
        ones1 = const.tile([1, 1], f32, name="ones1")
        nc.vector.memset(ones1, 1.0)

        # HAM pre-warm: give the PE ~5us of LDWEIGHTS busy-work while the
        # x DMA lands, so the activity monitor un-gates the clock before the
        # first projection matmul.
        for _w in range(N_WARM):
            nc.tensor.ldweights(weights=warm)

        # Phase 0: weights + x DMA ordered so projection group 0 starts as
        # early as possible on the serial DMA queue
        nc.sync.dma_start(out=wq, in_=wq_d[:].rearrange("(c p) d -> p c d", p=128))
        nc.sync.dma_start(out=wk, in_=wk_d[:].rearrange("(c p) d -> p c d", p=128))
        for c in range(NCH):
            nc.sync.dma_start(out=xT[:, c, 0:512], in_=x_d[c, :, 0:512])
        nc.sync.dma_start(out=wv, in_=wv_d[:].rearrange("(c p) d -> p c d", p=128))
        nc.sync.dma_start(out=wo, in_=wo_d[:])
        slabs = [(512, 1024), (1024, 2048)] + [
            (t0, t0 + 2048) for t0 in range(2048, TOK, 2048)
        ]
        for t0, t1 in slabs:
            for c in range(NCH):
                nc.sync.dma_start(
                    out=xT[:, c, t0:t1],
                    in_=x_d[c, :, t0:t1],
                )

        # ---- projection emitters ---------------------------------------
        def emit_proj_qk(g):
            """col-tiled q|k projection for query group g + layout copies"""
            b, qg, t0 = g // (N // NQ), g % (N // NQ), g * NQ
            qk = ps_sm.tile([128, NQ], f32, tag="o", name="qk")
            for c in range(NCH):
                nc.tensor.matmul(
                    qk[0:64, :], lhsT=wq[:, c, :], rhs=xT[:, c, t0 : t0 + NQ],
                    start=(c == 0), stop=(c == NCH - 1),
                )
                nc.tensor.matmul(
                    qk[64:128, :], lhsT=wk[:, c, :], rhs=xT[:, c, t0 : t0 + NQ],
                    start=(c == 0), stop=(c == NCH - 1),
                )
            nc.vector.tensor_copy(out=qT2[0:64, t0 : t0 + NQ], in_=qk[0:64, :])
            nc.vector.tensor_copy(out=qT2[64:128, t0 : t0 + NQ], in_=qk[0:64, :])
            kv = qk[64:128, :].rearrange("d (x c k) -> d c x k", x=2, c=2, k=128)
            for par in range(2):
                nc.vector.tensor_copy(
                    out=kT2[par * 64 : par * 64 + 64, b,
                            (2 * qg) * 128 : (2 * qg) * 128 + 256],
                    in_=kv[:, par, :, :],
                )

        def emit_proj_v(tb):
            """v projection for one 128-token block"""
            vp = ps_sm.tile([128, DH], f32, tag="o", name="vp")
            for c in range(NCH):
                nc.tensor.matmul(
                    vp, lhsT=xT[:, c, tb * 128 : tb * 128 + 128], rhs=wv[:, c, :],
                    start=(c == 0), stop=(c == NCH - 1),
                )
            nc.vector.tensor_copy(out=vP[:, tb, 0:DH], in_=vp)

        def emit_proj_group(g):
            emit_proj_qk(g)
            for t in range(NQ // 128):
                emit_proj_v(g * (NQ // 128) + t)

        # proj groups 0-3 are the serial prefix; the rest become pieces
        # interleaved into the attention stream (each piece = one psum tile)
        for g in range(4):
            emit_proj_group(g)
        pieces = []
        for g in range(4, NG):
            pieces.append(lambda g=g: emit_proj_qk(g))
            for t in range(NQ // 128):
                pieces.append(
                    lambda tb=g * (NQ // 128) + t: emit_proj_v(tb)
                )
        piece_i = 0

        # Phase 2+3: attention + output projection.
        #
        # oN holds the UNNORMALIZED attention output (bf16); the softmax
        # denominator (o row DH) is transposed into token-partition layout
        # via tiny K=1 matmuls, reciprocal'd wide, and applied as a
        # per-partition scalar fused into the projection's PSUM->SBUF copy.
        # Each group's epilogue (denominator transpose + projection) is
        # interleaved into the NEXT group's score loop so the in-order PE
        # queue never stalls on the normalize chain.
        def emit_denT(pq0, pden):
            denT = ps_sm.tile([128, 4], f32, tag="o", name="denT")
            for t in range(NQ // 128):
                nc.tensor.matmul(
                    denT[:, t : t + 1],
                    lhsT=pden[0:1, t * 128 : (t + 1) * 128],
                    rhs=ones1, start=True, stop=True,
                )
            recT = sb_io.tile([128, 4], f32, name="recT")
            nc.vector.reciprocal(recT, denT)
            return recT

        def emit_fp_one(pq0, t, recT):
            tt0 = pq0 + t * 128
            fp = ps_sm.tile([128, D], f32, tag="o", name="fp")
            nc.tensor.matmul(
                fp, lhsT=oN[:, tt0 : tt0 + 128], rhs=wo, start=True, stop=True
            )
            ob = sb_io.tile([128, D], f32, name="ob")
            nc.vector.tensor_scalar_mul(ob, in0=fp, scalar1=recT[:, t : t + 1])
            nc.sync.dma_start(out=out_d[tt0 : tt0 + 128, :], in_=ob)

        def emit_qk_score(s, b, blk, i, q0):
            # one score matmul for batch-local key block blk into s[:, i, :];
            # parity selects the PE row-half (row-tiled, K=64)
            par = blk % 2
            pr = blk >> 1
            nc.tensor.matmul(
                s[:, i, :],
                lhsT=kT2[par * 64 : par * 64 + 64, b, pr * 128 : pr * 128 + 128],
                rhs=qT2[par * 64 : par * 64 + 64, q0 : q0 + NQ],
                start=True, stop=True,
            )

        pending = None   # (q0, den tile) of the previous group
        pv_queue = None  # (o tile, b, p tile, jb, gsz) awaiting emission;
        #                  survives group boundaries so the last chunk's PV
        #                  overlaps the next group's first QK instead of
        #                  stalling on the final exp.

        def flush_pv(pv):
            po, pb, p, pjb, pgsz = pv
            for i in range(pgsz):
                jbg = pb * NJB + pjb + i
                nc.tensor.matmul(
                    po, lhsT=vP[:, jbg, :], rhs=p[:, i, :],
                    start=(pjb + i == 0), stop=(pjb + i == NJB - 1),
                )

        for gidx in range(NG):
            b, qg = gidx // (N // NQ), gidx % (N // NQ)
            q0 = b * N + qg * NQ
            o = None
            jb = 0
            recT = None
            for gi, gsz in enumerate(JGS):
                s = ps_s.tile([128, 3, NQ], f32, tag="s", name="s")
                # QK: one row-tiled even/odd pair + (maybe) one single.
                # The single is emitted LAST; on odd chunks it occupies PE
                # rows 64-127, so the previous group's epilogue matmuls
                # (fp: K=64 rows 0-63, denT: K=1 rows 0-31) emitted right
                # after it run concurrently with it.
                if jb % 2 == 0:
                    emit_qk_score(s, b, jb, 0, q0)
                    emit_qk_score(s, b, jb + 1, 1, q0)
                    if gsz == 3:
                        emit_qk_score(s, b, jb + 2, 2, q0)
                else:
                    emit_qk_score(s, b, jb + 1, 1, q0)
                    emit_qk_score(s, b, jb + 2, 2, q0)
                    emit_qk_score(s, b, jb, 0, q0)
                # previous group's fp projection, paired with the odd single
                if pending is not None and gi in (3, 5, 7, 9):
                    emit_fp_one(pending[0], (gi - 3) // 2, recT)
                if gi == 0:
                    # previous group's tail: flush its last PV chunk (overlaps
                    # this group's QK on the PE) and emit its oN/den copies
                    if pv_queue is not None:
                        flush_pv(pv_queue)
                        pv_queue = None
                        po, pq0 = pending_o
                        nc.vector.tensor_copy(
                            out=oN[:, pq0 : pq0 + NQ], in_=po[0:DH, :]
                        )
                        den = sb_io.tile([1, NQ], f32, name="den")
                        nc.vector.tensor_copy(out=den, in_=po[DH : DH + 1, :])
                        pending = (pq0, den)
                    o = ps_sm.tile([DH + 1, NQ], f32, tag="o", name="o")
                p = sb_p.tile([128, 3, NQ], bf16, name="p")
                # exp split: DVE (Schraudolph bit-trick) takes one EARLY
                # pair-member block (its scores land at chunk start, so the
                # 691ns DVE op finishes a full chunk before its PV consumer);
                # ScalarE takes the other two as a contiguous slice.  The
                # engines read different PSUM banks.
                nsc = 2 if gsz == 3 else gsz
                nc.scalar.activation(
                    out=p[:, 0:nsc, :], in_=s[:, 0:nsc, :], func=EXP, scale=SCALE
                )
                # interleaved projection piece #1 (covered by the QK matmuls)
                if piece_i < len(pieces):
                    pieces[piece_i]()
                    piece_i += 1
                if pv_queue is not None:
                    flush_pv(pv_queue)
                # piece #2 after the PV flush so consecutive pieces' psum-slot
                # reuse waits are covered by the PV matmuls
                if piece_i < len(pieces):
                    pieces[piece_i]()
                    piece_i += 1
                # DVE (Schraudolph bit-trick) exps block 2.  Emitted AFTER
                # the PV flush: the bitcast makes this an imprecise writer of
                # the p pool, so any PV matmul emitted after it waits for it
                # conservatively — after the flush, that ordering coincides
                # with the true dependency (only the NEXT flush needs it).
                if gsz == 3:
                    nc.vector.tensor_scalar(
                        out=p[:, 2, :].bitcast(i16), in0=s[:, 2, :],
                        scalar1=SCH_A, scalar2=SCH_B, op0=MULT, op1=ADD,
                    )
                pv_queue = (o, b, p, jb, gsz)
                jb += gsz
                # previous group's denominator transpose: after the PV flush
                # so its wait on the DVE den copy is covered by PE work
                if pending is not None and gi == 1:
                    recT = emit_denT(*pending)
            pending_o = (o, q0)

        flush_pv(pv_queue)
        po, pq0 = pending_o
        nc.vector.tensor_copy(out=oN[:, pq0 : pq0 + NQ], in_=po[0:DH, :])
        den = sb_io.tile([1, NQ], f32, name="den")
        nc.vector.tensor_copy(out=den, in_=po[DH : DH + 1, :])
        pending = (pq0, den)

        recT = emit_denT(*pending)
        for t in range(NQ // 128):
            emit_fp_one(pending[0], t, recT)

    nc.compile()
    return nc


def make_in_maps(x, Wq, Wk, Wv, Wo):
    bf16 = ml_dtypes.bfloat16
    # x transposed to [feat, tok] and chunked: [NCH, 128, TOK]
    x_bf = np.ascontiguousarray(
        x.reshape(TOK, D).T.reshape(NCH, 128, TOK)
    ).astype(bf16)
    in_maps = []
    for h in range(H):
        sl = slice(h * DH, (h + 1) * DH)
        in_maps.append(
            {
                "x": x_bf,
                "wq": np.ascontiguousarray(Wq[sl, :].T).astype(bf16),
                "wk": np.ascontiguousarray(Wk[sl, :].T).astype(bf16),
                "wv": np.ascontiguousarray(Wv[sl, :].T).astype(bf16),
                "wo": np.ascontiguousarray(Wo[:, sl].T).astype(bf16),
            }
        )
    return in_maps


def _install_ntff_shim():
    """The axon boot skips registering the NTFF profile hook when the image's
    antenv lacks axon_hooks; register an equivalent shim so trace=True works."""
    import types

    if "antenv.axon_hooks" in sys.modules:
        return
    try:
        from trn_agent_boot.trn_boot import _ntff_profile_via_ctypes

        hook = _ntff_profile_via_ctypes("/opt/axon/libaxon_pjrt.so")
    except Exception:
        hook = None
    mod = types.ModuleType("antenv.axon_hooks")
    mod.get_axon_ntff_profile_hook = lambda: hook
    sys.modules["antenv.axon_hooks"] = mod


def run(x, Wq, Wk, Wv, Wo, bo, trace=False):
    from concourse.bass_utils import run_bass_kernel_spmd

    if trace:
        _install_ntff_shim()

    nc = build_bass()
    in_maps = make_in_maps(x, Wq, Wk, Wv, Wo)
    res = run_bass_kernel_spmd(nc, in_maps, core_ids=list(range(H)), trace=trace)
    acc = np.zeros((TOK, D), dtype=np.float32)
    for r in res.results:
        acc += r["out"]
    acc += np.asarray(bo, dtype=np.float32)[None, :]
    return acc.reshape(B, N, D), res


def kernel(x, Wq, Wk, Wv, Wo, bo):
    out, _ = run(
        np.asarray(x, dtype=np.float32),
        np.asarray(Wq, dtype=np.float32),
        np.asarray(Wk, dtype=np.float32),
        np.asarray(Wv, dtype=np.float32),
        np.asarray(Wo, dtype=np.float32),
        np.asarray(bo, dtype=np.float32),
    )
    return out
